# revision 13
# baseline (speedup 1.0000x reference)
import os
import sys
from contextlib import ExitStack

import numpy as np

for _p in ("/opt/trn_rl_repo",):
    if os.path.isdir(_p) and _p not in sys.path:
        sys.path.insert(0, _p)

# Problem (nn_PosDecoder): out[n,l] = sum_c src[n,l,:128] . (table[1+c]*sqrt(128))
#   = src[n,l,:128] . colsum  where colsum = sqrt(128) * sum(table[1:], axis=0).
# Shard table rows across 8 cores; each core computes a partial colsum and a
# partial (N,L) output row; host sums the 8 partial rows.
#
# Host relayout: core k's 12500 rows -> 97 blocks of (128,128) -> 11 wide
# chunks of 8 blocks + 2 narrow chunks of 4 + 1 single block, f-major inside
# (col = f*nb + b) so each on-device fold is a contiguous innermost reduce.
# No DVE adds at all: every chunk is DMA'd to its own SBUF tile and folded
# (128,F*nb)->(128,F) as it lands; PE accumulates all the (128,F) parts (plus
# the 84-row tail and the single block) into colsum via one PSUM group.
# Colsum is permutation/associativity invariant, so any grouping is valid.
N, L, M = 16, 100, 256
F = 128
N_LOC = 100001
N_CORES = 8
R = (N_LOC - 1) // N_CORES  # 12500 table rows per core
TOK = N * L  # 1600
NBLK = R // 128  # 97 full 128-row blocks
TAIL = R - NBLK * 128  # 84
SCALE = float(np.sqrt(F))

CHUNK_PLAN = [(i * 8, 8) for i in range(11)] + [(88, 4), (92, 4), (96, 1)]
OFFS = np.cumsum([0] + [nb * F for _, nb in CHUNK_PLAN]).tolist()
WIDE_W = 8 * F  # 1024
NARROW_W = 4 * F  # 512

_BUILT = None


def _build():
    import concourse.bass as bass
    import concourse.tile as tile
    from concourse import bacc, mybir

    nc = bacc.Bacc("TRN2", target_bir_lowering=False, debug=False,
                   num_devices=N_CORES)
    f32 = mybir.dt.float32
    tabX = nc.dram_tensor("tabX", (128, NBLK * F), f32,
                          kind="ExternalInput").ap()
    tail84 = nc.dram_tensor("tail84", (TAIL, F), f32,
                            kind="ExternalInput").ap()
    srcT = nc.dram_tensor("srcT", (F, TOK), f32, kind="ExternalInput").ap()
    out = nc.dram_tensor("out", (1, TOK), f32, kind="ExternalOutput").ap()

    def seg(i):
        return tabX[:, OFFS[i]:OFFS[i + 1]]

    with tile.TileContext(nc) as tc, ExitStack() as ctx:
        sb = ctx.enter_context(tc.tile_pool(name="sb", bufs=1))
        schunks = ctx.enter_context(tc.tile_pool(name="schunks", bufs=4))
        achunks = ctx.enter_context(tc.tile_pool(name="achunks", bufs=4))
        parts = ctx.enter_context(tc.tile_pool(name="parts", bufs=3))
        psum1 = ctx.enter_context(
            tc.tile_pool(name="psum1", bufs=1, space=bass.MemorySpace.PSUM))
        psumv = ctx.enter_context(
            tc.tile_pool(name="psumv", bufs=4, space=bass.MemorySpace.PSUM))

        ones = sb.tile([128, 1], f32)
        nc.gpsimd.memset(ones[:], SCALE)  # folds the sqrt(F) scale into colsum
        srcT_sb = sb.tile([128, TOK], f32)
        out_sb = sb.tile([1, TOK], f32)
        g0 = sb.tile([128, F], f32)
        tailt = sb.tile([TAIL, F], f32)
        n0 = sb.tile([128, NARROW_W], f32)
        n1 = sb.tile([128, NARROW_W], f32)

        # --- sync queue: W1,W3,..,W11 then N0 then srcT[0:512]
        s_tiles = []
        for ci in (1, 3, 5, 7, 9, 11):
            ch = schunks.tile([128, WIDE_W], f32)
            nc.sync.dma_start(ch[:], seg(ci - 1))
            s_tiles.append(ch)
        nc.sync.dma_start(n0[:], seg(11))
        nc.sync.dma_start(srcT_sb[:, 0:512], srcT[:, 0:512])

        # --- act queue: W2,W4,..,W10, tail, g0, srcT rest, N1 last
        a_tiles = []
        for ci in (2, 4, 6, 8, 10):
            ch = achunks.tile([128, WIDE_W], f32)
            nc.scalar.dma_start(ch[:], seg(ci - 1))
            a_tiles.append(ch)
        nc.scalar.dma_start(tailt[:], tail84[:, :])
        nc.scalar.dma_start(g0[:], seg(13))
        nc.scalar.dma_start(srcT_sb[:, 512:1024], srcT[:, 512:1024])
        nc.scalar.dma_start(srcT_sb[:, 1024:1600], srcT[:, 1024:1600])
        nc.scalar.dma_start(n1[:], seg(12))

        # --- DVE folds in arrival order; PE accumulates each part into one
        #     PSUM group (tail opens it, part_N1 closes it).
        cps = psum1.tile([128, 1], f32)
        nc.tensor.matmul(cps[:], tailt[:], ones[:TAIL, :], start=True,
                         stop=False)

        def fold_and_mm(src_tile):
            pt = parts.tile([128, F], f32)
            nc.vector.tensor_reduce(
                pt[:], src_tile.rearrange("p (f b) -> p f b", f=F),
                axis=mybir.AxisListType.X, op=mybir.AluOpType.add)
            nc.tensor.matmul(cps[:], pt[:], ones[:], start=False, stop=False)

        order = []
        for i in range(6):
            order.append(s_tiles[i])
            if i < 5:
                order.append(a_tiles[i])
        for ch in order:
            fold_and_mm(ch)
        nc.tensor.matmul(cps[:], g0[:], ones[:], start=False, stop=False)
        fold_and_mm(n0)
        ptN = parts.tile([128, F], f32)
        nc.vector.tensor_reduce(
            ptN[:], n1.rearrange("p (f b) -> p f b", f=F),
            axis=mybir.AxisListType.X, op=mybir.AluOpType.add)
        nc.tensor.matmul(cps[:], ptN[:], ones[:], start=False, stop=True)
        colsum = sb.tile([128, 1], f32)
        nc.vector.tensor_copy(colsum[:], cps[:])

        # --- out_row = colsum^T @ srcT -> (1, 1600); per-slice copy + DMA out
        for j in range(0, TOK, 512):
            w = min(512, TOK - j)
            pv = psumv.tile([1, 512], f32)
            nc.tensor.matmul(pv[:1, :w], colsum[:], srcT_sb[:, j:j + w],
                             start=True, stop=True)
            nc.vector.tensor_copy(out_sb[:, j:j + w], pv[:1, :w])
            nc.sync.dma_start(out[:, j:j + w], out_sb[:, j:j + w])

    nc.compile()
    return nc


def make_in_maps(src, lookup_table):
    src_f = np.asarray(src, dtype=np.float32).reshape(TOK, M)[:, :F]
    srcT_np = np.ascontiguousarray(src_f.T)  # (128, 1600)
    tab = np.asarray(lookup_table, dtype=np.float32)
    in_maps = []
    for k in range(N_CORES):
        sl = tab[1 + k * R:1 + (k + 1) * R, :]
        blocks = sl[:NBLK * 128].reshape(128, NBLK, F)  # [p, t, f]
        segs = []
        for b0, nb in CHUNK_PLAN:
            sub = blocks[:, b0:b0 + nb, :]  # [p, b, f] -> f-major [p, f, b]
            segs.append(sub.transpose(0, 2, 1).reshape(128, F * nb))
        tabX = np.ascontiguousarray(np.concatenate(segs, axis=1))
        tail_np = np.ascontiguousarray(sl[NBLK * 128:])
        in_maps.append({"tabX": tabX, "tail84": tail_np, "srcT": srcT_np})
    return in_maps


def kernel(src=None, ds=None, lookup_table=None, **_):
    global _BUILT
    if _BUILT is None:
        _BUILT = _build()
    from concourse import bass_utils

    in_maps = make_in_maps(src, lookup_table)
    res = bass_utils.run_bass_kernel_spmd(_BUILT, in_maps,
                                          core_ids=list(range(N_CORES)))
    parts = [next(iter(r.values())).reshape(-1) for r in res.results]
    total = np.sum(np.stack(parts, 0), axis=0, dtype=np.float64)
    return total.astype(np.float32).reshape(N, L)
